# revision 1
# baseline (speedup 1.0000x reference)
"""Sliding-window GQA attention block (RoPE + QKV proj + SDPA + O proj) on 8
Trainium2 NeuronCores, head-sharded (1 kv-head group = 8 q-heads per core).

Contract: kernel(**inputs) takes the FULL unsharded inputs from
setup_inputs() and returns the FULL [1, 2048, 2880] output.

Per-core plan (core c owns q-heads [8c, 8c+8), kv-head c):
  - host passes x.T (padded, with a ones-row to fold biases into the matmul),
    per-core transposed weight slices, RoPE cos/sin tables (head-dim order
    permuted so the rotate-half partner is the adjacent partition, making the
    RoPE "rotate" a within-quadrant stream_shuffle), and additive mask tiles.
  - QKV projections as fp32r matmuls (stream 512-wide), RoPE in the PSUM
    epilogue, producing qT/kT in [head_dim, seq] layout (fp32r) and v in
    natural [seq, head_dim] layout (bf16, via PE transposes) with an
    appended all-ones block so the attention-value matmul also produces the
    softmax denominators (replicated across 64 partitions).
  - Attention in S^T layout: per key-tile j, scores.T [128 keys, 256 queries]
    (one fp32r matmul), additive sliding-window mask, exp (scale=1/8 folded),
    then bf16 AV matmuls accumulate out.T + denominators per query tile.
    Normalization = one reciprocal + one multiply per (head, query-tile).
  - O projection: fp32r matmuls over the 4 attn-out partition tiles,
    streaming wo.T; per-core partial [2048, 2880] returned to host.
  - host sums the 8 partials and adds wo_b.
"""
import sys

sys.path.insert(0, "/opt/trn_rl_repo")

import numpy as np

import concourse.bass as bass  # noqa: F401  (import keeps bass registered)
import concourse.tile as tile
from concourse import bacc, mybir
from concourse.bass_utils import run_bass_kernel_spmd

B, S, D = 1, 2048, 2880
H, KVH, HD = 64, 8, 64
WINDOW = 128
N_CORES = 8
DP = 2944  # padded contraction dim: 23 * 128 (2880 data + 1 ones row + pad)
KT = DP // 128  # 23 contraction tiles
NQT = S // 128  # 16 seq tiles
OCH = 480  # O-proj free chunk (6 * 480 = 2880)

F32R = mybir.dt.float32r
F32 = mybir.dt.float32
BF16 = mybir.dt.bfloat16

# head-dim permutation: pairs (t, t+32) adjacent -> rotate-half partner is
# the neighbouring partition (stream_shuffle mask i^1 within quadrants)
PERM = np.empty(HD, dtype=np.int64)
PERM[0::2] = np.arange(32)
PERM[1::2] = np.arange(32) + 32

_COMPILED = None


def _build(debug=False):
    nc = bacc.Bacc("TRN2", target_bir_lowering=False, debug=False)

    xT_d = nc.dram_tensor("xT", [DP, S], F32R, kind="ExternalInput").ap()
    wq_d = nc.dram_tensor("wq", [DP, 512], F32R, kind="ExternalInput").ap()
    wkv_d = nc.dram_tensor("wkv", [DP, 128], F32R, kind="ExternalInput").ap()
    wo_d = nc.dram_tensor("wo", [512, D], F32R, kind="ExternalInput").ap()
    cos_d = nc.dram_tensor("cosT", [128, S], F32, kind="ExternalInput").ap()
    sin_d = nc.dram_tensor("sinTs", [128, S], F32, kind="ExternalInput").ap()
    ma0_d = nc.dram_tensor("ma0", [128, 256], F32, kind="ExternalInput").ap()
    ma1_d = nc.dram_tensor("ma1", [128, 128], F32, kind="ExternalInput").ap()
    id_d = nc.dram_tensor("id64", [64, 64], BF16, kind="ExternalInput").ap()
    out_d = nc.dram_tensor("partial", [S, D], F32, kind="ExternalOutput").ap()
    if debug:
        dbg_qT_d = nc.dram_tensor("dbg_qT", [128, S], F32, kind="ExternalOutput").ap()
        dbg_kT_d = nc.dram_tensor("dbg_kT", [128, S], F32, kind="ExternalOutput").ap()
        dbg_vx_d = nc.dram_tensor("dbg_vx", [128, 128], F32, kind="ExternalOutput").ap()
        dbg_ao_d = nc.dram_tensor("dbg_ao", [128, S], F32, kind="ExternalOutput").ap()

    Exp = mybir.ActivationFunctionType.Exp
    SHUF_MASK = [i ^ 1 for i in range(32)]

    with tile.TileContext(nc) as tc:
        with (
            tc.tile_pool(name="constp", bufs=1) as constp,
            tc.tile_pool(name="qkvp", bufs=1) as qkvp,
            tc.tile_pool(name="vextp", bufs=1) as vextp,
            tc.tile_pool(name="workp", bufs=3) as workp,
        ):
            cos_t = constp.tile([128, S], F32)
            sin_t = constp.tile([128, S], F32)
            ma0_t = constp.tile([128, 256], F32)
            ma1_t = constp.tile([128, 128], F32)
            id_t = constp.tile([64, 64], BF16)
            nc.sync.dma_start(cos_t[:], cos_d[:])
            nc.sync.dma_start(sin_t[:], sin_d[:])
            nc.sync.dma_start(ma0_t[:], ma0_d[:])
            nc.sync.dma_start(ma1_t[:], ma1_d[:])
            nc.sync.dma_start(id_t[:], id_d[:])

            # qTm[mt] holds heads (2mt, 2mt+1); kT2 holds kT duplicated in both
            # partition halves so scores lhsT/rhs base partitions match.
            qTm = [qkvp.tile([128, S], F32R, name=f"qTm{t}") for t in range(4)]
            kT2 = qkvp.tile([128, S], F32R, name="kT2")
            vT = qkvp.tile([64, S], BF16, name="vT")
            v_ext = [vextp.tile([128, 128], BF16, name=f"vx{i}") for i in range(NQT)]

            # ---------------- Phase 1: QKV projections + RoPE ----------------
            with (
                tc.tile_pool(name="wpool", bufs=1) as wpool,
                tc.tile_pool(name="xsp", bufs=4) as xsp,
                tc.tile_pool(name="psq", bufs=6, space="PSUM") as psq,
            ):
                wq_ts = []
                wkv_ts = []
                for k in range(KT):
                    wq_t = wpool.tile([128, 512], F32R, name=f"wq{k}")
                    wkv_t = wpool.tile([128, 128], F32R, name=f"wkv{k}")
                    nc.sync.dma_start(wq_t[:], wq_d[128 * k : 128 * (k + 1), :])
                    nc.sync.dma_start(wkv_t[:], wkv_d[128 * k : 128 * (k + 1), :])
                    wq_ts.append(wq_t)
                    wkv_ts.append(wkv_t)

                for sq in range(4):
                    c0 = 512 * sq
                    psums = [
                        psq.tile([128, 512], F32, name="psq_t", tag="psq_t")
                        for _ in range(5)
                    ]
                    xq = []
                    for k in range(KT):
                        x_t = xsp.tile([128, 512], F32R, name="xq_t")
                        nc.sync.dma_start(
                            x_t[:], xT_d[128 * k : 128 * (k + 1), c0 : c0 + 512]
                        )
                        xq.append(x_t)
                    for k in range(KT):
                        for mt in range(4):
                            nc.tensor.matmul(
                                psums[mt][:],
                                wq_ts[k][:, 128 * mt : 128 * (mt + 1)],
                                xq[k][:],
                                start=(k == 0),
                                stop=(k == KT - 1),
                            )
                        nc.tensor.matmul(
                            psums[4][:],
                            wkv_ts[k][:],
                            xq[k][:],
                            start=(k == 0),
                            stop=(k == KT - 1),
                        )
                    # RoPE epilogues: q m-tiles (2 heads each)
                    for mt in range(4):
                        ps = psums[mt]
                        t_all = workp.tile([128, 512], F32, tag="ra")
                        nc.scalar.copy(t_all[:], ps[:])
                        t_shuf = workp.tile([128, 512], F32, tag="rb")
                        nc.vector.stream_shuffle(t_shuf[:], t_all[:], SHUF_MASK)
                        t_cos = workp.tile([128, 512], F32, tag="rc")
                        nc.vector.tensor_mul(t_cos[:], t_all[:], cos_t[:, c0 : c0 + 512])
                        t_sin = workp.tile([128, 512], F32, tag="rd")
                        nc.vector.tensor_mul(t_sin[:], t_shuf[:], sin_t[:, c0 : c0 + 512])
                        nc.vector.tensor_add(
                            qTm[mt][:, c0 : c0 + 512], t_cos[:], t_sin[:]
                        )
                    # kv epilogue: k rope (rows 0:64) + v copy (rows 64:128)
                    ps = psums[4]
                    t_allk = workp.tile([128, 512], F32, tag="ra", name="t_allk")
                    nc.scalar.copy(t_allk[0:64, :], ps[0:64, :])
                    t_shufk = workp.tile([128, 512], F32, tag="rb", name="t_shufk")
                    nc.vector.stream_shuffle(t_shufk[0:64, :], t_allk[0:64, :], SHUF_MASK)
                    t_cosk = workp.tile([128, 512], F32, tag="rc", name="t_cosk")
                    nc.vector.tensor_mul(
                        t_cosk[0:64, :], t_allk[0:64, :], cos_t[0:64, c0 : c0 + 512]
                    )
                    t_sink = workp.tile([128, 512], F32, tag="rd", name="t_sink")
                    nc.vector.tensor_mul(
                        t_sink[0:64, :], t_shufk[0:64, :], sin_t[0:64, c0 : c0 + 512]
                    )
                    nc.vector.tensor_add(
                        kT2[0:64, c0 : c0 + 512], t_cosk[0:64, :], t_sink[0:64, :]
                    )
                    nc.vector.tensor_add(
                        kT2[64:128, c0 : c0 + 512], t_cosk[0:64, :], t_sink[0:64, :]
                    )
                    nc.vector.tensor_copy(vT[:, c0 : c0 + 512], ps[64:128, :])

                if debug:
                    dq = workp.tile([128, 512], F32, tag="ra", name="dq")
                    for sq4 in range(4):
                        nc.vector.tensor_copy(dq[:], qTm[0][:, 512*sq4:512*(sq4+1)])
                        nc.sync.dma_start(dbg_qT_d[:, 512*sq4:512*(sq4+1)], dq[:])
                        dk = workp.tile([128, 512], F32, tag="rb", name="dk")
                        nc.vector.tensor_copy(dk[:], kT2[:, 512*sq4:512*(sq4+1)])
                        nc.sync.dma_start(dbg_kT_d[:, 512*sq4:512*(sq4+1)], dk[:])
                # v transposes -> v_ext natural layout + ones block
                for i in range(NQT):
                    tr = psq.tile([128, 64], BF16, name="vtr", tag="vtr", bufs=2)
                    nc.tensor.transpose(tr[:], vT[:, 128 * i : 128 * (i + 1)], id_t[:])
                    nc.vector.tensor_copy(v_ext[i][:, 0:64], tr[:])
                    nc.vector.memset(v_ext[i][:, 64:128], 1.0)

            if debug:
                dvx = workp.tile([128, 128], F32, tag="sm", name="dvx")
                nc.vector.tensor_copy(dvx[:], v_ext[2][:])
                nc.sync.dma_start(dbg_vx_d[:], dvx[:])
            # ------------- Phase 2: attention + O-projection, per seq tile ----
            with (
                tc.tile_pool(name="aoutp", bufs=1) as aoutp,
                tc.tile_pool(name="wosp", bufs=1) as wosp,
                tc.tile_pool(name="epool", bufs=18) as epool,
                tc.tile_pool(name="outsp", bufs=3) as outsp,
                tc.tile_pool(name="psS", bufs=2, space="PSUM") as psS,
                tc.tile_pool(name="psO", bufs=4, space="PSUM") as psO,
                tc.tile_pool(name="psP", bufs=2, space="PSUM") as psP,
            ):
                attn_oT = [aoutp.tile([128, S], F32R, name=f"aoT{t}") for t in range(4)]
                wo_sb = []
                for t in range(4):
                    w_t = wosp.tile([128, D], F32R, name=f"wo{t}")
                    nc.sync.dma_start(w_t[:], wo_d[128 * t : 128 * (t + 1), :])
                    wo_sb.append(w_t)

                e_prev = None

                for j in range(NQT):
                    W = 256 if j < 15 else 128
                    e_cur = []
                    for h in range(8):
                        rb = 64 * (h % 2)
                        pss = psS.tile([128, 256], F32, name="pss", tag="pss")
                        nc.tensor.matmul(
                            pss[:, 0:W],
                            kT2[rb : rb + 64, 128 * j : 128 * (j + 1)],
                            qTm[h // 2][rb : rb + 64, 128 * j : 128 * j + W],
                            start=True,
                            stop=True,
                        )
                        s_m = workp.tile([128, 256], F32, tag="sm")
                        nc.vector.tensor_add(
                            s_m[:, 0:W],
                            pss[:, 0:W],
                            ma0_t[:, 0:W] if j < 15 else ma1_t[:],
                        )
                        e_t = epool.tile([128, 256], BF16, tag="e")
                        nc.scalar.activation(e_t[:, 0:W], s_m[:, 0:W], Exp, scale=0.125)
                        e_cur.append(e_t)
                    # AV + denominators for qtile j: keys from tiles j-1 and j,
                    # contiguous 2-matmul accumulation group per head
                    po = [
                        psO.tile([128, 512], F32, name="po", tag="po")
                        for _ in range(2)
                    ]
                    for h in range(8):
                        g, hh = h // 4, h % 4
                        if j > 0:
                            nc.tensor.matmul(
                                po[g][:, 128 * hh : 128 * (hh + 1)],
                                v_ext[j - 1][:],
                                e_prev[h][:, 128:256],
                                start=True,
                                stop=False,
                            )
                        nc.tensor.matmul(
                            po[g][:, 128 * hh : 128 * (hh + 1)],
                            v_ext[j][:],
                            e_cur[h][:, 0:128],
                            start=(j == 0),
                            stop=True,
                        )
                    # normalize qtile j -> attn_oT
                    for h in range(8):
                        g, hh = h // 4, h % 4
                        pgo = po[g]
                        rec = workp.tile([64, 128], F32, tag="rec")
                        nc.vector.reciprocal(
                            rec[:], pgo[64:128, 128 * hh : 128 * (hh + 1)]
                        )
                        t, rb = h // 2, 64 * (h % 2)
                        nc.vector.tensor_mul(
                            attn_oT[t][rb : rb + 64, 128 * j : 128 * (j + 1)],
                            pgo[0:64, 128 * hh : 128 * (hh + 1)],
                            rec[:],
                        )
                    e_prev = e_cur
                    if debug:
                        dao = workp.tile([128, 128], F32, tag="sm", name="dao")
                        nc.vector.tensor_copy(dao[:], attn_oT[0][:, 128*j:128*(j+1)])
                        nc.sync.dma_start(dbg_ao_d[:, 128*j:128*(j+1)], dao[:])
                    # O-projection for seq tile j
                    for ch in range(6):
                        pp = psP.tile([128, OCH], F32, name="pp", tag="pp")
                        for t in range(4):
                            nc.tensor.matmul(
                                pp[:],
                                attn_oT[t][:, 128 * j : 128 * (j + 1)],
                                wo_sb[t][:, OCH * ch : OCH * (ch + 1)],
                                start=(t == 0),
                                stop=(t == 3),
                            )
                        osb = outsp.tile([128, OCH], F32, tag="osb")
                        nc.any.tensor_copy(osb[:], pp[:])
                        nc.sync.dma_start(
                            out_d[128 * j : 128 * (j + 1), OCH * ch : OCH * (ch + 1)],
                            osb[:],
                        )
    nc.compile()
    return nc


def _prep_inputs(x, rope_cache, wq_w, wq_b, wk_w, wk_b, wv_w, wv_b, wo_w):
    """Build the shared + per-core input maps."""
    xT = np.zeros((DP, S), dtype=np.float32)
    xT[0:D, :] = np.ascontiguousarray(x[0].T)
    xT[D, :] = 1.0  # bias row

    cos = np.asarray(rope_cache[:, 0, :], dtype=np.float32)  # [S, 64]
    sin = np.asarray(rope_cache[:, 1, :], dtype=np.float32)
    cosP = cos[:, PERM].T  # [64, S] permuted head-dim rows
    sinP = sin[:, PERM].T
    sign = np.where(PERM < 32, -1.0, 1.0).astype(np.float32)[:, None]
    sinPs = sinP * sign
    cosT = np.concatenate([cosP, cosP], axis=0).astype(np.float32)  # [128, S]
    sinTs = np.concatenate([sinPs, sinPs], axis=0).astype(np.float32)

    b_idx = np.arange(128)[:, None]
    a_idx = np.arange(256)[None, :]
    ma0 = np.where((b_idx <= a_idx) & (a_idx < b_idx + WINDOW), 0.0, -1e30).astype(
        np.float32
    )
    ma1 = np.where(b_idx <= a_idx[:, :128], 0.0, -1e30).astype(np.float32)
    import ml_dtypes

    id64 = np.eye(64, dtype=np.float32).astype(ml_dtypes.bfloat16)

    shared = dict(xT=xT, cosT=cosT, sinTs=sinTs, ma0=ma0, ma1=ma1, id64=id64)

    in_maps = []
    for c in range(N_CORES):
        # wq slice: q heads [8c, 8c+8), head-dim permuted, transposed, bias row
        wq_rows = []
        bq_rows = []
        for hh in range(8):
            g = 8 * c + hh
            wq_rows.append(wq_w[64 * g + PERM, :])  # [64, D]
            bq_rows.append(wq_b[64 * g + PERM])
        wq_slice = np.concatenate(wq_rows, axis=0)  # [512, D]
        bq_slice = np.concatenate(bq_rows, axis=0)  # [512]
        wq_t = np.zeros((DP, 512), dtype=np.float32)
        wq_t[0:D, :] = wq_slice.T
        wq_t[D, :] = bq_slice

        wk_slice = wk_w[64 * c + PERM, :]  # [64, D] permuted
        bk_slice = wk_b[64 * c + PERM]
        wv_slice = wv_w[64 * c : 64 * (c + 1), :]  # unpermuted
        bv_slice = wv_b[64 * c : 64 * (c + 1)]
        wkv_t = np.zeros((DP, 128), dtype=np.float32)
        wkv_t[0:D, 0:64] = wk_slice.T
        wkv_t[0:D, 64:128] = wv_slice.T
        wkv_t[D, 0:64] = bk_slice
        wkv_t[D, 64:128] = bv_slice

        wo_t = np.ascontiguousarray(wo_w[:, 512 * c : 512 * (c + 1)].T).astype(
            np.float32
        )  # [512, D]

        in_maps.append(dict(shared, wq=wq_t, wkv=wkv_t, wo=wo_t))
    return in_maps


def kernel(
    x,
    rope_cache,
    wq_w,
    wq_b,
    wk_w,
    wk_b,
    wv_w,
    wv_b,
    wo_w,
    wo_b,
):
    global _COMPILED
    x = np.asarray(x, dtype=np.float32)
    rope_cache = np.asarray(rope_cache, dtype=np.float32)
    wq_w = np.asarray(wq_w, dtype=np.float32)
    wq_b = np.asarray(wq_b, dtype=np.float32)
    wk_w = np.asarray(wk_w, dtype=np.float32)
    wk_b = np.asarray(wk_b, dtype=np.float32)
    wv_w = np.asarray(wv_w, dtype=np.float32)
    wv_b = np.asarray(wv_b, dtype=np.float32)
    wo_w = np.asarray(wo_w, dtype=np.float32)
    wo_b = np.asarray(wo_b, dtype=np.float32)

    if _COMPILED is None:
        _COMPILED = _build()
    nc = _COMPILED

    in_maps = _prep_inputs(x, rope_cache, wq_w, wq_b, wk_w, wk_b, wv_w, wv_b, wo_w)
    res = run_bass_kernel_spmd(nc, in_maps, core_ids=list(range(N_CORES)), trace=False)
    out = np.zeros((S, D), dtype=np.float32)
    for c in range(N_CORES):
        out += res.results[c]["partial"]
    out += wo_b[None, :]
    return out.reshape(B, S, D).astype(np.float32)


# expose the compiled-module runner for test harnesses that want tracing
def run_traced(**inputs):
    global _COMPILED
    if _COMPILED is None:
        _COMPILED = _build()
    in_maps = _prep_inputs(
        np.asarray(inputs["x"], np.float32),
        np.asarray(inputs["rope_cache"], np.float32),
        np.asarray(inputs["wq_w"], np.float32),
        np.asarray(inputs["wq_b"], np.float32),
        np.asarray(inputs["wk_w"], np.float32),
        np.asarray(inputs["wk_b"], np.float32),
        np.asarray(inputs["wv_w"], np.float32),
        np.asarray(inputs["wv_b"], np.float32),
        np.asarray(inputs["wo_w"], np.float32),
    )
    res = run_bass_kernel_spmd(
        _COMPILED, in_maps, core_ids=list(range(N_CORES)), trace=True
    )
    out = np.zeros((S, D), dtype=np.float32)
    for c in range(N_CORES):
        out += res.results[c]["partial"]
    out += np.asarray(inputs["wo_b"], np.float32)[None, :]
    return out.reshape(B, S, D).astype(np.float32), res



# revision 16
# speedup vs baseline: 1.1689x; 1.1689x over previous
"""Sliding-window GQA attention block (RoPE + QKV proj + SDPA + O proj) on 8
Trainium2 NeuronCores, head-sharded (1 kv-head group = 8 q-heads per core).

Contract: kernel(**inputs) takes the FULL unsharded inputs from
setup_inputs() and returns the FULL [1, 2048, 2880] output.

v2 design (single-phase pipeline, bf16 datapath, S-layout softmax):
  - All device inputs are bf16 (xT with a ones-row folding biases into the
    matmul, per-core transposed weight slices packed as [DP, 640] = q|k|v,
    RoPE cos/sin with head-dim permutation + sign folded, 0/1 masks).
  - QKV projections stream per 512-seq chunk, mt-major (one PSUM bank per
    output m-tile, rotating through a shared 4-bank pool); RoPE epilogue on
    DVE in bf16; qT/kT kept in [head_dim, seq] bf16; v transposed to natural
    [seq, head_dim] via PE.
  - Attention per 128-query tile j in S layout: scores [q, 256 keys]
    (keys = tiles j-1, j) with the additive -1e30 window mask folded in as
    a second matmul (id128^T @ mask) into the same PSUM accumulation; the
    scalar engine then does exp straight from PSUM with accum_out giving
    the softmax denominator [128,1] per head; one reciprocal [128,8] per
    tile; normalize e, transpose it on the PE, AV accumulates the
    normalized attention output [hd, q] directly.
  - O-projection of tile j-1 is emitted between tile j's scores and its
    normalize/AV stage: it fills the PE while the exp/mask chain drains,
    keeping the tensor engine continuously busy (full-clock p-state).
  - PSUM: shared tag (qkv accum + v-transposes + score pairs + eT) 4 banks,
    attention-out po 2 banks, O-proj 2 banks = 8 banks exactly.
  - Per-core partial [2048, 2880] fp32 returned; host sums 8 partials + bias.
"""
import sys

sys.path.insert(0, "/opt/trn_rl_repo")

import numpy as np

import concourse.bass as bass  # noqa: F401  (import keeps bass registered)
import concourse.tile as tile
from concourse import bacc, mybir
from concourse.bass_utils import run_bass_kernel_spmd

B, S, D = 1, 2048, 2880
H, KVH, HD = 64, 8, 64
WINDOW = 128
N_CORES = 8
DP = 2944  # padded contraction dim: 23 * 128 (2880 data + 1 ones row + pad)
KT = DP // 128  # 23 contraction tiles
NQT = S // 128  # 16 seq tiles
OCH = 480  # O-proj free chunk (6 * 480 = 2880)

F32 = mybir.dt.float32
BF16 = mybir.dt.bfloat16

# head-dim permutation: pairs (t, t+32) adjacent -> rotate-half partner is
# the neighbouring partition (stream_shuffle mask i^1 within quadrants)
PERM = np.empty(HD, dtype=np.int64)
PERM[0::2] = np.arange(32)
PERM[1::2] = np.arange(32) + 32

_COMPILED = None


def _build():
    nc = bacc.Bacc("TRN2", target_bir_lowering=False, debug=False)

    xT_d = nc.dram_tensor("xT", [DP, S], BF16, kind="ExternalInput").ap()
    w_d = nc.dram_tensor("wqkv", [DP, 640], BF16, kind="ExternalInput").ap()
    wo_d = nc.dram_tensor("wo", [512, D], BF16, kind="ExternalInput").ap()
    cos_d = nc.dram_tensor("cosT", [128, S], BF16, kind="ExternalInput").ap()
    sin_d = nc.dram_tensor("sinTs", [128, S], BF16, kind="ExternalInput").ap()
    mm_d = nc.dram_tensor("m_main", [128, 256], BF16, kind="ExternalInput").ap()
    md_d = nc.dram_tensor("m_diag", [128, 128], BF16, kind="ExternalInput").ap()
    id64_d = nc.dram_tensor("id64", [64, 64], BF16, kind="ExternalInput").ap()
    id128_d = nc.dram_tensor("id128", [128, 128], BF16, kind="ExternalInput").ap()
    out_d = nc.dram_tensor("partial", [S, D], F32, kind="ExternalOutput").ap()

    Exp = mybir.ActivationFunctionType.Exp
    MULT = mybir.AluOpType.mult
    ADD = mybir.AluOpType.add
    SHUF_MASK = [i ^ 1 for i in range(32)]

    with tile.TileContext(nc) as tc:
        with (
            tc.tile_pool(name="constp", bufs=1) as constp,
            tc.tile_pool(name="wpool", bufs=1) as wpool,
            tc.tile_pool(name="xsp", bufs=46) as xsp,
            tc.tile_pool(name="qkvp", bufs=1) as qkvp,
            tc.tile_pool(name="workp", bufs=3) as workp,
            tc.tile_pool(name="epool", bufs=3) as epool,
            tc.tile_pool(name="empool", bufs=10) as empool,
            tc.tile_pool(name="drp", bufs=3) as drp,
            tc.tile_pool(name="outsp", bufs=8) as outsp,
            tc.tile_pool(name="psQS", bufs=4, space="PSUM") as psQS,
            tc.tile_pool(name="psPO", bufs=2, space="PSUM") as psPO,
            tc.tile_pool(name="psPP", bufs=2, space="PSUM") as psPP,
        ):
            # ---- constants ----
            cos_t = constp.tile([128, S], BF16)
            sin_t = constp.tile([128, S], BF16)
            mm_t = constp.tile([128, 256], BF16)
            md_t = constp.tile([128, 128], BF16)
            id64_t = constp.tile([64, 64], BF16)
            id128_t = constp.tile([128, 128], BF16)
            nc.sync.dma_start(cos_t[:], cos_d[:])
            nc.sync.dma_start(sin_t[:], sin_d[:])
            nc.sync.dma_start(mm_t[:], mm_d[:])
            nc.sync.dma_start(md_t[:], md_d[:])
            nc.sync.dma_start(id64_t[:], id64_d[:])
            nc.sync.dma_start(id128_t[:], id128_d[:])

            # ---- weight + first x chunk DMAs, issue-interleaved so the
            # first contraction tiles land first ----
            w_sb = []
            x_sb = [[None] * KT for _ in range(4)]
            for k in range(KT):
                w_t = wpool.tile([128, 640], BF16, name=f"w{k}")
                nc.sync.dma_start(w_t[:], w_d[128 * k : 128 * (k + 1), :])
                w_sb.append(w_t)
                x_t = xsp.tile([128, 512], BF16, name="x_t", tag="x")
                nc.sync.dma_start(x_t[:], xT_d[128 * k : 128 * (k + 1), 0:512])
                x_sb[0][k] = x_t
            wo_sb = []
            for t in range(4):
                w_t = wpool.tile([128, D], BF16, name=f"wo{t}")
                nc.sync.dma_start(w_t[:], wo_d[128 * t : 128 * (t + 1), :])
                wo_sb.append(w_t)

            def prefetch_x(sq):
                for k in range(KT):
                    x_t = xsp.tile([128, 512], BF16, name="x_t", tag="x")
                    nc.sync.dma_start(
                        x_t[:], xT_d[128 * k : 128 * (k + 1), 512 * sq : 512 * (sq + 1)]
                    )
                    x_sb[sq][k] = x_t

            # ---- persistent bf16 activation tensors ----
            qTm = [qkvp.tile([128, S], BF16, name=f"qTm{t}") for t in range(4)]
            kT2 = qkvp.tile([128, S], BF16, name="kT2")
            vT = qkvp.tile([64, S], BF16, name="vT")
            # v duplicated into both column halves so the AV matmul output
            # covers both partition halves -> all attn copies are shift-free
            v_sb = [qkvp.tile([128, 128], BF16, name=f"v{i}") for i in range(NQT)]
            attn_oT = [qkvp.tile([128, S], BF16, name=f"aoT{t}") for t in range(4)]

            def rope_q(ps, mt, c0):
                t_all = workp.tile([128, 512], BF16, tag="ra", name="t_all")
                nc.scalar.copy(t_all[:], ps[:])
                t_shuf = workp.tile([128, 512], BF16, tag="rb", name="t_shuf")
                nc.vector.stream_shuffle(t_shuf[:], t_all[:], SHUF_MASK)
                t_cos = workp.tile([128, 512], BF16, tag="rc", name="t_cos")
                nc.vector.tensor_mul(t_cos[:], t_all[:], cos_t[:, c0 : c0 + 512])
                t_sin = workp.tile([128, 512], BF16, tag="rd", name="t_sin")
                nc.vector.tensor_mul(t_sin[:], t_shuf[:], sin_t[:, c0 : c0 + 512])
                nc.vector.tensor_add(qTm[mt][:, c0 : c0 + 512], t_cos[:], t_sin[:])

            def rope_kv(ps, c0):
                t_all = workp.tile([128, 512], BF16, tag="ra", name="t_allk")
                nc.scalar.copy(t_all[0:64, :], ps[0:64, :])
                t_shuf = workp.tile([128, 512], BF16, tag="rb", name="t_shufk")
                nc.vector.stream_shuffle(t_shuf[0:64, :], t_all[0:64, :], SHUF_MASK)
                t_cos = workp.tile([128, 512], BF16, tag="rc", name="t_cosk")
                nc.vector.tensor_mul(
                    t_cos[0:64, :], t_all[0:64, :], cos_t[0:64, c0 : c0 + 512]
                )
                t_sin = workp.tile([128, 512], BF16, tag="rd", name="t_sink")
                nc.vector.tensor_mul(
                    t_sin[0:64, :], t_shuf[0:64, :], sin_t[0:64, c0 : c0 + 512]
                )
                nc.vector.tensor_add(
                    kT2[0:64, c0 : c0 + 512], t_cos[0:64, :], t_sin[0:64, :]
                )
                nc.vector.tensor_add(
                    kT2[64:128, c0 : c0 + 512], t_cos[0:64, :], t_sin[0:64, :]
                )
                nc.vector.tensor_copy(vT[:, c0 : c0 + 512], ps[64:128, :])

            def emit_oproj(j):
                for ch in range(6):
                    pp = psPP.tile([128, OCH], F32, name="pp", tag="pp")
                    for t in range(4):
                        nc.tensor.matmul(
                            pp[:],
                            attn_oT[t][:, 128 * j : 128 * (j + 1)],
                            wo_sb[t][:, OCH * ch : OCH * (ch + 1)],
                            start=(t == 0),
                            stop=(t == 3),
                        )
                    osb = outsp.tile([128, OCH], F32, tag="osb", name="osb")
                    # split PSUM->SBUF copies between the two aux engines
                    eng = nc.vector if ch % 2 == 0 else nc.scalar
                    if eng is nc.scalar:
                        nc.scalar.copy(osb[:], pp[:])
                    else:
                        nc.vector.tensor_copy(osb[:], pp[:])
                    nc.sync.dma_start(
                        out_d[128 * j : 128 * (j + 1), OCH * ch : OCH * (ch + 1)],
                        osb[:],
                    )

            for sq in range(4):
                c0 = 512 * sq
                if sq < 3:
                    prefetch_x(sq + 1)
                # ---- QKV projections, mt-major ----
                for mt in range(4):
                    ps = psQS.tile([128, 512], F32, tag="qs", name="ps_q")
                    for k in range(KT):
                        nc.tensor.matmul(
                            ps[:],
                            w_sb[k][:, 128 * mt : 128 * (mt + 1)],
                            x_sb[sq][k][:],
                            start=(k == 0),
                            stop=(k == KT - 1),
                        )
                    rope_q(ps, mt, c0)
                ps = psQS.tile([128, 512], F32, tag="qs", name="ps_kv")
                for k in range(KT):
                    nc.tensor.matmul(
                        ps[:],
                        w_sb[k][:, 512:640],
                        x_sb[sq][k][:],
                        start=(k == 0),
                        stop=(k == KT - 1),
                    )
                rope_kv(ps, c0)
                # ---- v transposes to natural [seq, hd], duplicated halves ----
                for i in range(4 * sq, 4 * sq + 4):
                    tr = psQS.tile([128, 128], BF16, tag="qs", name="vtr")
                    nc.tensor.transpose(
                        tr[:, 0:64], vT[:, 128 * i : 128 * (i + 1)], id64_t[:]
                    )
                    nc.tensor.transpose(
                        tr[:, 64:128], vT[:, 128 * i : 128 * (i + 1)], id64_t[:]
                    )
                    nc.scalar.copy(v_sb[i][:], tr[:])

                # ---- attention for the 4 query tiles of this chunk ----
                for j in range(4 * sq, 4 * sq + 4):
                    W = 128 if j == 0 else 256
                    kc0 = 0 if j == 0 else 128 * (j - 1)
                    mask = md_t if j == 0 else mm_t
                    den = drp.tile([128, 8], F32, tag="den", name="den")
                    e_ms = []
                    s_pair = None
                    for h in range(8):
                        t, rb = h // 2, 64 * (h % 2)
                        if h % 2 == 0:
                            s_pair = psQS.tile(
                                [128, 512], F32, tag="qs", name="s_pair"
                            )
                        sl = s_pair[:, 256 * (h % 2) : 256 * (h % 2) + W]
                        nc.tensor.matmul(
                            sl,
                            qTm[t][rb : rb + 64, 128 * j : 128 * (j + 1)],
                            kT2[rb : rb + 64, kc0 : kc0 + W],
                            start=True,
                            stop=False,
                        )
                        # additive -1e30 window mask via id128^T @ mask
                        nc.tensor.matmul(
                            sl,
                            id128_t[:],
                            mask[:, 0:W],
                            start=False,
                            stop=True,
                        )
                        # masked exp straight from PSUM; accum_out = softmax den
                        e_m = empool.tile([128, 256], BF16, tag="em", name="e_m")
                        nc.scalar.activation(
                            e_m[:, 0:W],
                            sl,
                            Exp,
                            scale=0.125,
                            accum_out=den[:, h : h + 1],
                        )
                        e_ms.append(e_m)
                    # PE filler: O-projection of the previous query tile
                    if j > 0:
                        emit_oproj(j - 1)
                    rec = drp.tile([128, 8], F32, tag="rec", name="rec")
                    nc.vector.reciprocal(rec[:], den[:])
                    po = [
                        psPO.tile([128, 512], F32, tag="po", name="po")
                        for _ in range(2)
                    ]
                    eT_ps = None
                    eT_sb = None
                    for h in range(8):
                        g, hh = h // 4, h % 4
                        hp = h % 2  # position within the eT pair
                        e_n = epool.tile([128, 256], BF16, tag="en", name="e_n")
                        nc.vector.tensor_scalar_mul(
                            e_n[:, 0:W], e_ms[h][:, 0:W], rec[:, h : h + 1]
                        )
                        if hp == 0:
                            eT_ps = psQS.tile([128, 512], BF16, tag="qs", name="eT_ps")
                        for half in range(W // 128):
                            nc.tensor.transpose(
                                eT_ps[:, 256 * hp + 128 * half : 256 * hp + 128 * (half + 1)],
                                e_n[:, 128 * half : 128 * (half + 1)],
                                id128_t[:],
                            )
                        if hp == 1:
                            eT_sb = epool.tile(
                                [128, 512], BF16, tag="et", name="eT_sb"
                            )
                            if W == 256:
                                nc.vector.tensor_copy(eT_sb[:], eT_ps[:])
                            else:
                                nc.vector.tensor_copy(
                                    eT_sb[:, 0:128], eT_ps[:, 0:128]
                                )
                                nc.vector.tensor_copy(
                                    eT_sb[:, 256:384], eT_ps[:, 256:384]
                                )
                            for h2 in (h - 1, h):
                                g2, hh2 = h2 // 4, h2 % 4
                                hp2 = h2 % 2
                                # AV: left half = keys tile j-1, right = tile j
                                if j > 0:
                                    nc.tensor.matmul(
                                        po[g2][:, 128 * hh2 : 128 * (hh2 + 1)],
                                        v_sb[j - 1][:],
                                        eT_sb[:, 256 * hp2 : 256 * hp2 + 128],
                                        start=True,
                                        stop=False,
                                    )
                                    nc.tensor.matmul(
                                        po[g2][:, 128 * hh2 : 128 * (hh2 + 1)],
                                        v_sb[j][:],
                                        eT_sb[:, 256 * hp2 + 128 : 256 * hp2 + 256],
                                        start=False,
                                        stop=True,
                                    )
                                else:
                                    nc.tensor.matmul(
                                        po[g2][:, 128 * hh2 : 128 * (hh2 + 1)],
                                        v_sb[0][:],
                                        eT_sb[:, 256 * hp2 : 256 * hp2 + 128],
                                        start=True,
                                        stop=True,
                                    )
                    for h in range(8):
                        t, rb = h // 2, 64 * (h % 2)
                        g, hh = h // 4, h % 4
                        # po rows are duplicated halves -> no partition shift
                        nc.scalar.copy(
                            attn_oT[t][rb : rb + 64, 128 * j : 128 * (j + 1)],
                            po[g][rb : rb + 64, 128 * hh : 128 * (hh + 1)],
                        )
            emit_oproj(NQT - 1)
    nc.compile()
    return nc


def _prep_inputs(x, rope_cache, wq_w, wq_b, wk_w, wk_b, wv_w, wv_b, wo_w):
    """Build the shared + per-core input maps (all bf16 on device)."""
    import ml_dtypes

    bf = ml_dtypes.bfloat16
    xT = np.zeros((DP, S), dtype=np.float32)
    xT[0:D, :] = np.ascontiguousarray(x[0].T)
    xT[D, :] = 1.0  # bias row
    xT = xT.astype(bf)

    cos = np.asarray(rope_cache[:, 0, :], dtype=np.float32)  # [S, 64]
    sin = np.asarray(rope_cache[:, 1, :], dtype=np.float32)
    cosP = cos[:, PERM].T  # [64, S] permuted head-dim rows
    sinP = sin[:, PERM].T
    sign = np.where(PERM < 32, -1.0, 1.0).astype(np.float32)[:, None]
    sinPs = sinP * sign
    cosT = np.concatenate([cosP, cosP], axis=0).astype(bf)  # [128, S]
    sinTs = np.concatenate([sinPs, sinPs], axis=0).astype(bf)

    # S-layout additive masks: query a on partitions, key b on free.
    a_idx = np.arange(128)[:, None]
    b_idx = np.arange(256)[None, :]
    left = (b_idx < 128) & (b_idx > a_idx)
    right = (b_idx >= 128) & ((b_idx - 128) <= a_idx)
    m_main = np.where(left | right, 0.0, -1e30).astype(bf)  # [128, 256], j >= 1
    m_diag = np.where(b_idx[:, :128] <= a_idx, 0.0, -1e30).astype(bf)  # j == 0

    id64 = np.eye(64, dtype=np.float32).astype(bf)
    id128 = np.eye(128, dtype=np.float32).astype(bf)

    shared = dict(
        xT=xT, cosT=cosT, sinTs=sinTs, m_main=m_main, m_diag=m_diag,
        id64=id64, id128=id128,
    )

    in_maps = []
    for c in range(N_CORES):
        # wq slice: q heads [8c, 8c+8), head-dim permuted, transposed, bias row
        wq_rows = []
        bq_rows = []
        for hh in range(8):
            g = 8 * c + hh
            wq_rows.append(wq_w[64 * g + PERM, :])  # [64, D]
            bq_rows.append(wq_b[64 * g + PERM])
        wq_slice = np.concatenate(wq_rows, axis=0)  # [512, D]
        bq_slice = np.concatenate(bq_rows, axis=0)  # [512]

        wk_slice = wk_w[64 * c + PERM, :]  # [64, D] permuted
        bk_slice = wk_b[64 * c + PERM]
        wv_slice = wv_w[64 * c : 64 * (c + 1), :]  # unpermuted
        bv_slice = wv_b[64 * c : 64 * (c + 1)]

        w_t = np.zeros((DP, 640), dtype=np.float32)
        w_t[0:D, 0:512] = wq_slice.T
        w_t[D, 0:512] = bq_slice
        w_t[0:D, 512:576] = wk_slice.T
        w_t[D, 512:576] = bk_slice
        w_t[0:D, 576:640] = wv_slice.T
        w_t[D, 576:640] = bv_slice

        wo_t = np.ascontiguousarray(
            wo_w[:, 512 * c : 512 * (c + 1)].T
        ).astype(bf)  # [512, D]

        in_maps.append(dict(shared, wqkv=w_t.astype(bf), wo=wo_t))
    return in_maps


def kernel(
    x,
    rope_cache,
    wq_w,
    wq_b,
    wk_w,
    wk_b,
    wv_w,
    wv_b,
    wo_w,
    wo_b,
):
    global _COMPILED
    x = np.asarray(x, dtype=np.float32)
    rope_cache = np.asarray(rope_cache, dtype=np.float32)
    wq_w = np.asarray(wq_w, dtype=np.float32)
    wq_b = np.asarray(wq_b, dtype=np.float32)
    wk_w = np.asarray(wk_w, dtype=np.float32)
    wk_b = np.asarray(wk_b, dtype=np.float32)
    wv_w = np.asarray(wv_w, dtype=np.float32)
    wv_b = np.asarray(wv_b, dtype=np.float32)
    wo_w = np.asarray(wo_w, dtype=np.float32)
    wo_b = np.asarray(wo_b, dtype=np.float32)

    if _COMPILED is None:
        _COMPILED = _build()
    nc = _COMPILED

    in_maps = _prep_inputs(x, rope_cache, wq_w, wq_b, wk_w, wk_b, wv_w, wv_b, wo_w)
    res = run_bass_kernel_spmd(nc, in_maps, core_ids=list(range(N_CORES)), trace=False)
    out = np.zeros((S, D), dtype=np.float32)
    for c in range(N_CORES):
        out += res.results[c]["partial"]
    out += wo_b[None, :]
    return out.reshape(B, S, D).astype(np.float32)


# expose the compiled-module runner for test harnesses that want tracing
def run_traced(**inputs):
    global _COMPILED
    if _COMPILED is None:
        _COMPILED = _build()
    in_maps = _prep_inputs(
        np.asarray(inputs["x"], np.float32),
        np.asarray(inputs["rope_cache"], np.float32),
        np.asarray(inputs["wq_w"], np.float32),
        np.asarray(inputs["wq_b"], np.float32),
        np.asarray(inputs["wk_w"], np.float32),
        np.asarray(inputs["wk_b"], np.float32),
        np.asarray(inputs["wv_w"], np.float32),
        np.asarray(inputs["wv_b"], np.float32),
        np.asarray(inputs["wo_w"], np.float32),
    )
    res = run_bass_kernel_spmd(
        _COMPILED, in_maps, core_ids=list(range(N_CORES)), trace=True
    )
    out = np.zeros((S, D), dtype=np.float32)
    for c in range(N_CORES):
        out += res.results[c]["partial"]
    out += np.asarray(inputs["wo_b"], np.float32)[None, :]
    return out.reshape(B, S, D).astype(np.float32), res
